# revision 1
# baseline (speedup 1.0000x reference)
"""CPAMDec attention-decoder kernel for 8 Trainium2 NeuronCores.

Reference computation (per batch n of N=8):
    q  = x_n^T @ wq^T + bq          (HW=4096, C4=128)
    k  = y_n @ wk^T + bk            (K=32, C4=128)
    v  = y_n @ wv^T + bv            (K=32, C=512)
    attn = softmax(q @ k^T, axis=-1)        (HW, K)
    out = scale * (v^T @ attn^T) + x_n      (C, HW)

Sharding: pure data parallel — core i computes batch i. Params are
replicated (host pre-transposes them so no on-device transposes are
needed). Heavy matmuls run in float32r (1 cycle/row at N=512); the
residual add reads the original fp32 bits of x, so the dominant output
term is exact.

Structure: column-streaming pipeline. x arrives in 8 chunks of 512
pixels (one strided DMA each); each chunk runs the full chain
q -> energy -> exp -> sum -> 1/sum -> attn -> out-matmul -> +bv+x -> out
so input DMA, compute on all engines, and output DMA overlap. PSUM is
partitioned per stage (q:2, e:2, s:2, o:2 banks) for cross-chunk
double buffering.

Bias folding:
  - bq contributes a per-key bias e_b[j] = sum_o bq[o]*k[j,o], applied
    inside the exp() activation (exact algebra).
  - bv enters as a per-partition scalar in the fused output op
    osb = (o_ps + s*bv[c]) + x, using sum_j attn[p,j] = 1.
"""

import sys

sys.path.insert(0, "/opt/trn_rl_repo")

import numpy as np

import concourse.bacc as bacc
import concourse.mybir as mybir
import concourse.tile as tile
from concourse.alu_op_type import AluOpType
from concourse.bass_utils import run_bass_kernel_spmd

F32 = mybir.dt.float32
F32R = mybir.dt.float32r
AF = mybir.ActivationFunctionType

N, C, H, W, K = 8, 512, 64, 64, 32
HW = H * W            # 4096
C4 = C // 4           # 128
PC = 512              # free-dim chunk (1 PSUM bank of fp32)
NPC = HW // PC        # 8 chunks
KC = C // 128         # 4 contraction chunks
CT = C // 128         # 4 output row-tiles


def _load_consts(nc, tc, cst, cdma):
    """Constant loads on the sync HWDGE ring (wq first — q(0) needs it)."""
    yt, wq, wkt, wv = [], [], [], []
    for k in range(KC):
        t = cst.tile([128, C4], F32R, name=f"wq{k}", tag=f"wq{k}")
        cdma.dma_start(t[:], nc.t.wqT[k * 128:(k + 1) * 128, :].bitcast(F32R))
        wq.append(t)
    for k in range(KC):
        t = cst.tile([128, 4 * K], F32R, name=f"yt{k}", tag=f"yt{k}")
        cdma.dma_start(t[:], nc.t.yT[k * 128:(k + 1) * 128, :].bitcast(F32R))
        yt.append(t)
    for k in range(KC):
        t = cst.tile([128, C4], F32R, name=f"wk{k}", tag=f"wk{k}")
        cdma.dma_start(t[:], nc.t.wkT[k * 128:(k + 1) * 128, :].bitcast(F32R))
        wkt.append(t)

    wv = []
    for k in range(KC):
        t = cst.tile([128, C], F32R, name=f"wv{k}", tag=f"wv{k}")
        cdma.dma_start(t[:], nc.t.wvT[k * 128:(k + 1) * 128, :].bitcast(F32R))
        wv.append(t)
    bq_r = cst.tile([C4, K], F32R, name="bq_r", tag="bq_r")
    cdma.dma_start(bq_r[:], nc.t.bqb[:].bitcast(F32R))
    bk_sb = cst.tile([C4, 1], F32, name="bk_sb", tag="bk_sb")
    cdma.dma_start(bk_sb[:], nc.t.bk[:])
    bvt_sb = cst.tile([C4, CT], F32, name="bvt_sb", tag="bvt_sb")
    cdma.dma_start(bvt_sb[:], nc.t.bvt[:])
    s_bc32 = cst.tile([K, 1], F32, name="s_bc32", tag="s_bc32")
    nc.gpsimd.dma_start(
        s_bc32[:], nc.t.s[:].partition_broadcast(K).squeeze(-1))
    s_bc128 = cst.tile([C4, 1], F32, name="s_bc128", tag="s_bc128")
    nc.gpsimd.dma_start(
        s_bc128[:], nc.t.s[:].partition_broadcast(C4).squeeze(-1))
    ones32 = cst.tile([K, 128], F32R, name="ones32", tag="ones32")
    nc.gpsimd.dma_start(
        ones32[:],
        nc.t.ones[0:1, 0:128].bitcast(F32R).partition_broadcast(K).squeeze(1))
    return yt, wq, wkt, wv, bq_r, bk_sb, bvt_sb, s_bc32, s_bc128, ones32


def _emit(nc, tc):
    sync = nc.sync
    cdma = sync  # constants share the sync HWDGE ring (idle early)

    with (
        tc.tile_pool(name="const", bufs=1) as cst,
        tc.tile_pool(name="xbuf", bufs=1) as xp,
        tc.tile_pool(name="work", bufs=3) as wk_pool,
        tc.tile_pool(name="ps", bufs=2, space="PSUM") as ps,
    ):
        # x column chunks: (128 part, 4 c-tiles, 512 cols) strided loads
        xs = [None] * NPC

        def load_chunk(pc):
            t = xp.tile([128, KC, PC], F32R, name=f"xs{pc}", tag=f"xs{pc}")
            src = nc.t.x[:, pc * PC:(pc + 1) * PC].bitcast(F32R).rearrange(
                "(k p) f -> p k f", p=128)
            nc.scalar.dma_start(t[:], src)
            xs[pc] = t

        # prefetch the first two chunks before anything else hits the ring
        load_chunk(0)
        load_chunk(1)

        # PE warm-up: the HAM clock gate only unthrottles (1.2 -> 2.4 GHz)
        # after ~3.4us of *sustained* matmul activity, and it re-throttles
        # only after ~3.4us of continuous idle. The PE sits idle until x
        # arrives (~13us) — burn that window with dummy matmuls so every
        # real matmul afterwards runs at full clock.
        dmy_w = cst.tile([128, 128], F32R, name="dmy_w", tag="dmy_w")
        sync.dma_start(dmy_w[:], nc.t.x[0:128, 0:128].bitcast(F32R))
        dmy_x = cst.tile([128, PC], F32R, name="dmy_x", tag="dmy_x")
        sync.dma_start(dmy_x[:], nc.t.x[0:128, 0:PC].bitcast(F32R))
        dmy_ps = ps.tile([128, PC], F32, name="dmy_ps", tag="q", bufs=2)
        for _ in range(18):
            nc.tensor.matmul(dmy_ps[:], dmy_w[:], dmy_x[:],
                             start=True, stop=True)

        (yt, wq, wkt, wv, bq_r, bk_sb, bvt_sb, s_bc32, s_bc128,
         ones32) = _load_consts(nc, tc, cst, cdma)

        pro = {}

        def emit_prologue():
            # kT (with bk), v (scaled by s), e_b, s*bv — emitted after
            # stage_q(0) so q(0) leads the PE queue.
            kt_ps = ps.tile([C4, 4 * K], F32, name="kt_ps", tag="e", bufs=1)
            for k in range(KC):
                nc.tensor.matmul(kt_ps[:], wkt[k][:], yt[k][:],
                                 start=(k == 0), stop=(k == KC - 1))
            ktb4 = cst.tile([C4, 4 * K], F32R, name="ktb4", tag="ktb4")
            nc.scalar.activation(out=ktb4[:], in_=kt_ps[:], func=AF.Identity,
                                 bias=bk_sb[:], scale=1.0)

            v_ps = ps.tile([K, C], F32, name="v_ps", tag="s", bufs=1)
            for k in range(KC):
                nc.tensor.matmul(v_ps[:], yt[k][:, 0:K], wv[k][:],
                                 start=(k == 0), stop=(k == KC - 1))
            v_sb = cst.tile([K, C], F32R, name="v_sb", tag="v_sb")
            nc.scalar.activation(out=v_sb[:], in_=v_ps[:], func=AF.Copy,
                                 bias=0.0, scale=s_bc32[:])
            # partition-stacked copy for row-packed final matmuls:
            # vstack[32*ct + j, m] = v_sb[j, 128*ct + m]
            vstack = cst.tile([128, 128], F32R, name="vstack", tag="vstack")
            for ct in range(CT):
                nc.gpsimd.dma_start(
                    vstack[32 * ct:32 * (ct + 1), :],
                    v_sb[:, 128 * ct:128 * (ct + 1)])

            eb_ps = ps.tile([4 * K, K], F32, name="eb_ps", tag="o", bufs=4)
            nc.tensor.matmul(eb_ps[:], ktb4[:], bq_r[:], start=True,
                             stop=True)
            e_b4 = cst.tile([4 * K, 1], F32, name="e_b4", tag="e_b4")
            nc.scalar.activation(out=e_b4[:], in_=eb_ps[:, 0:1],
                                 func=AF.Copy, scale=1.0)

            bvs = cst.tile([C4, CT], F32, name="bvs", tag="bvs")
            nc.vector.tensor_scalar_mul(bvs[:], bvt_sb[:], s_bc128[:])
            pro.update(ktb4=ktb4, v_sb=v_sb, vstack=vstack, e_b4=e_b4,
                       bvs=bvs)

        # ------------- software-pipelined main loop over column chunks ----
        # Stages are skewed so every PE instruction's inputs were produced
        # in an earlier iteration (the engine queues are in-order; without
        # the skew the PE stalls mid-chain waiting on ACT/DVE/GpSimd).
        #   step:   q(step)   e/exp(step-1)   sum/rec/mul(step-2)
        #           finals/add/store(step-3)
        qtcs = [None] * NPC
        expts = [None] * NPC
        attns = [None] * NPC

        def stage_q(pc):
            xt = xs[pc]
            q_ps = ps.tile([C4, PC], F32, name=f"q_ps{pc}", tag="q", bufs=2)
            for k in range(KC):
                nc.tensor.matmul(q_ps[:], wq[k][:], xt[:, k, :],
                                 start=(k == 0), stop=(k == KC - 1))
            qtc = wk_pool.tile([C4, PC], F32R, name="qtc", tag="qtc", bufs=4)
            nc.scalar.activation(out=qtc[:], in_=q_ps[:], func=AF.Copy,
                                 scale=1.0)
            qtcs[pc] = qtc

        def stage_energy(pc):
            e_ps = ps.tile([128, PC], F32, name=f"e_ps{pc}", tag="e", bufs=1)
            nc.tensor.matmul(e_ps[:], pro['ktb4'][:], qtcs[pc][:],
                             start=True, stop=True)
            expt = wk_pool.tile([128, PC], F32R, name="expt", tag="expt",
                                bufs=4)
            nc.scalar.activation(out=expt[:], in_=e_ps[:], func=AF.Exp,
                                 bias=pro['e_b4'][:], scale=1.0)
            expts[pc] = expt

        def stage_softmax(pc):
            s_ps = ps.tile([128, PC], F32, name=f"s_ps{pc}", tag="s", bufs=1)
            nc.tensor.matmul(s_ps[:], ones32[:], expts[pc][0:K, :],
                             start=True, stop=True)
            rec = wk_pool.tile([128, PC], F32, name="rec", tag="rec", bufs=4)
            nc.vector.reciprocal_approx_fast(
                out=rec[:], in_=s_ps[:].bitcast(F32))
            attn = wk_pool.tile([128, PC], F32R, name="attn", tag="attn",
                                bufs=4)
            nc.vector.tensor_mul(attn[:], expts[pc][:].bitcast(F32), rec[:])
            attns[pc] = attn

        def stage_out(pc):
            sl = slice(pc * PC, (pc + 1) * PC)
            xt = xs[pc]
            attn = attns[pc]
            osb = wk_pool.tile([128, CT, PC], F32, name="osb", tag="osb",
                               bufs=3)
            for ct in range(CT):
                o_ps = ps.tile([128, PC], F32, name=f"o_ps{pc}_{ct}",
                               tag="o", bufs=4)
                nc.tensor.matmul(o_ps[:],
                                 pro['vstack'][32 * ct:32 * (ct + 1), :],
                                 attn[32 * ct:32 * (ct + 1), :],
                                 start=True, stop=True,
                                 tile_position=(32 * ct, 0))
                nc.vector.scalar_tensor_tensor(
                    out=osb[:, ct, :], in0=o_ps[:],
                    scalar=pro['bvs'][:, ct:ct + 1],
                    in1=xt[:, ct, :].bitcast(F32),
                    op0=AluOpType.add, op1=AluOpType.add)
            dst = nc.t.out[:, sl].rearrange("(k p) f -> p k f", p=128)
            sync.dma_start(dst, osb[:])

        stage_q(0)
        emit_prologue()
        for step in range(1, NPC + 3):
            if 2 <= step + 3 < NPC:
                load_chunk(step + 3)
            if step == 1:
                load_chunk(2)
                load_chunk(3)
            if step < NPC:
                stage_q(step)
            if 0 <= step - 1 < NPC:
                stage_energy(step - 1)
            if 0 <= step - 2 < NPC:
                stage_softmax(step - 2)
            if 0 <= step - 3 < NPC:
                stage_out(step - 3)


class _T:
    """Attribute access to declared dram params."""
    def __init__(self):
        self.__dict__ = {}


_NC_CACHE = []


def _build():
    if _NC_CACHE:
        return _NC_CACHE[0]
    nc = bacc.Bacc(target_bir_lowering=False)
    nc.t = _T()
    t = nc.t
    t.x = nc.declare_dram_parameter("x", [C, HW], F32, isOutput=False)
    t.yT = nc.declare_dram_parameter("yT", [C, 4 * K], F32, isOutput=False)
    t.wqT = nc.declare_dram_parameter("wqT", [C, C4], F32, isOutput=False)
    t.wkT = nc.declare_dram_parameter("wkT", [C, C4], F32, isOutput=False)
    t.wvT = nc.declare_dram_parameter("wvT", [C, C], F32, isOutput=False)
    t.bqb = nc.declare_dram_parameter("bqb", [C4, K], F32, isOutput=False)
    t.bk = nc.declare_dram_parameter("bk", [C4, 1], F32, isOutput=False)
    t.bvt = nc.declare_dram_parameter("bvt", [C4, CT], F32, isOutput=False)
    t.s = nc.declare_dram_parameter("s", [1, 1], F32, isOutput=False)
    t.ones = nc.declare_dram_parameter("ones", [1, HW], F32, isOutput=False)
    t.out = nc.declare_dram_parameter("out", [C, HW], F32, isOutput=True)
    with tile.TileContext(nc) as tc:
        _emit(nc, tc)
    nc.finalize()
    _NC_CACHE.append(nc)
    return nc


def _in_maps(x, y, wq, bq, wk, bk, wv, bv, scale):
    x = np.ascontiguousarray(x, dtype=np.float32).reshape(N, C, HW)
    yT = np.ascontiguousarray(
        np.tile(np.transpose(y, (0, 2, 1)), (1, 1, 4)), dtype=np.float32)
    wqT = np.ascontiguousarray(wq.T, dtype=np.float32)
    wkT = np.ascontiguousarray(wk.T, dtype=np.float32)
    wvT = np.ascontiguousarray(wv.T, dtype=np.float32)
    bqb = np.ascontiguousarray(
        np.broadcast_to(np.float32(bq).reshape(C4, 1), (C4, K)),
        dtype=np.float32)
    bk = np.ascontiguousarray(bk, dtype=np.float32).reshape(C4, 1)
    bvt = np.ascontiguousarray(
        np.float32(bv).reshape(CT, C4).T, dtype=np.float32)
    s = np.ascontiguousarray(scale, dtype=np.float32).reshape(1, 1)
    return [
        {
            "x": x[i], "yT": yT[i], "wqT": wqT, "wkT": wkT, "wvT": wvT,
            "bqb": bqb, "bk": bk, "bvt": bvt, "s": s,
            "ones": np.ones((1, HW), dtype=np.float32),
        }
        for i in range(N)
    ]


def _run(inputs, **kwargs):
    nc = _build()
    return run_bass_kernel_spmd(nc, _in_maps(**inputs),
                                core_ids=list(range(N)), **kwargs)


def kernel(**inputs) -> np.ndarray:
    res = _run(inputs)
    out = np.stack([res.results[i]["out"] for i in range(N)])
    return out.reshape(N, C, H, W).astype(np.float32)



# revision 3
# speedup vs baseline: 1.0960x; 1.0960x over previous
"""CPAMDec attention-decoder kernel for 8 Trainium2 NeuronCores.

Reference computation (per batch n of N=8):
    q  = x_n^T @ wq^T + bq          (HW=4096, C4=128)
    k  = y_n @ wk^T + bk            (K=32, C4=128)
    v  = y_n @ wv^T + bv            (K=32, C=512)
    attn = softmax(q @ k^T, axis=-1)        (HW, K)
    out = scale * (v^T @ attn^T) + x_n      (C, HW)

Sharding: pure data parallel - core i computes batch i; params replicated.

Key optimizations over a straightforward port:
  - bf16 I/O. x and out move over HBM as bf16 (8MB -> 4MB each way per
    core); HBM-per-NC (~358 GB/s) is the binding roofline. rel-err of the
    full bf16 pipeline is ~3e-3, well under the 2e-2 gate.
  - wq folding: e[j,p] = sum_c EM[c,j] x[c,p] with EM = wq^T @ (k^T+bk)
    computed once in the prologue. The per-chunk q stage (4 matmuls + an
    ACT copy) disappears; energy comes straight from x.
  - bq contributes a per-key bias e_b[j] = sum_o bq[o]*ktb[o,j], applied
    inside the exp() activation (exact algebra).
  - Residual add runs on the PE: o_ps = eye@x + sum_j s*v[j,c]*attn[j,p]
    accumulated in PSUM, so no vector op ever touches a second tensor.
  - bv enters as a per-partition bias in the PSUM->SBUF copy
    (ACT activation bias / DVE tensor_scalar_add), using sum_j attn = 1.
  - PE p-state management: the tensor engine only reaches 2.4 GHz after
    ~3us of gap-free execution and drops back on any idle gap. Warm-up
    dummies + per-chunk filler matmuls keep the run continuous.
"""

import sys

sys.path.insert(0, "/opt/trn_rl_repo")

import numpy as np
import ml_dtypes

import concourse.bacc as bacc
import concourse.mybir as mybir
import concourse.tile as tile
from concourse.alu_op_type import AluOpType
from concourse.bass_utils import run_bass_kernel_spmd

F32 = mybir.dt.float32
BF16 = mybir.dt.bfloat16
AF = mybir.ActivationFunctionType
BF = ml_dtypes.bfloat16

N, C, H, W, K = 8, 512, 64, 64, 32
HW = H * W            # 4096
C4 = C // 4           # 128
PC = 512              # free-dim chunk (1 PSUM bank of fp32)
NPC = HW // PC        # 8 chunks
KC = C // 128         # 4 contraction chunks
CT = C // 128         # 4 output row-tiles


def _emit(nc, tc):
    sync = nc.sync
    cdma = nc.scalar      # consts + stores ride the ACT HWDGE ring

    with (
        tc.tile_pool(name="const", bufs=1) as cst,
        tc.tile_pool(name="xbuf", bufs=1) as xp,
        tc.tile_pool(name="work", bufs=3) as wk_pool,
        tc.tile_pool(name="ps", bufs=2, space="PSUM") as ps,
    ):
        # ---------------- constant loads ----------------
        # warm-up data first (dummy matmuls need it ~immediately)
        dmy = cst.tile([128, PC], BF16, name="dmy", tag="dmy")
        cdma.dma_start(dmy[:], nc.t.x[0:128, 0:PC])
        bk_sb = cst.tile([C4, 1], F32, name="bk_sb", tag="bk_sb")
        cdma.dma_start(bk_sb[:], nc.t.bk[:])
        bvt_sb = cst.tile([C4, CT], F32, name="bvt_sb", tag="bvt_sb")
        cdma.dma_start(bvt_sb[:], nc.t.bvt[:])
        wkt = cst.tile([128, KC, C4], BF16, name="wkt", tag="wkt")
        cdma.dma_start(wkt[:], nc.t.wkt[:].rearrange("(k p) f -> p k f", p=128))
        yt = cst.tile([128, KC, 4 * K], BF16, name="yt", tag="yt")
        cdma.dma_start(yt[:], nc.t.yt[:].rearrange("(k p) f -> p k f", p=128))
        wqo = cst.tile([C4, C], BF16, name="wqo", tag="wqo")
        cdma.dma_start(wqo[:], nc.t.wqo[:])
        bqb = cst.tile([C4, K], BF16, name="bqb", tag="bqb")
        cdma.dma_start(bqb[:], nc.t.bqb[:])
        eye = cst.tile([128, 128], BF16, name="eye", tag="eye")
        cdma.dma_start(eye[:], nc.t.eye[:])
        wvt = cst.tile([128, KC, C], BF16, name="wvt", tag="wvt")
        cdma.dma_start(wvt[:], nc.t.wvt[:].rearrange("(k p) f -> p k f", p=128))

        # broadcast scale + ones on the gpsimd (SWDGE) path
        s_bc32 = cst.tile([K, 1], F32, name="s_bc32", tag="s_bc32")
        nc.gpsimd.dma_start(
            s_bc32[:], nc.t.s[:].partition_broadcast(K).squeeze(-1))
        s_bc128 = cst.tile([C4, 1], F32, name="s_bc128", tag="s_bc128")
        nc.gpsimd.dma_start(
            s_bc128[:], nc.t.s[:].partition_broadcast(C4).squeeze(-1))
        ones32 = cst.tile([K, 128], BF16, name="ones32", tag="ones32")
        nc.gpsimd.memset(ones32[:], 1.0)

        # x column chunks: (128 part, 4 c-tiles, PC cols) strided loads on
        # the SP ring. SBUF is plentiful: keep all 8 resident.
        xs = [None] * NPC

        def load_chunk(pc):
            t = xp.tile([128, KC, PC], BF16, name=f"xs{pc}", tag=f"xs{pc}")
            src = nc.t.x[:, pc * PC:(pc + 1) * PC].rearrange(
                "(k p) f -> p k f", p=128)
            sync.dma_start(t[:], src)
            xs[pc] = t

        for pc in range(4):
            load_chunk(pc)

        # ---------------- PE warm-up ----------------
        # Ramp the PE p-state (0.65 -> 1.2 -> 2.4 GHz after 3us of gap-free
        # execution) while DMAs land. dmy_ps is a dedicated PSUM bank so
        # fillers never create dependencies with real work.
        dmy_ps = ps.tile([128, PC], F32, name="dmy_ps", tag="dmy", bufs=1)
        for _ in range(8):
            nc.tensor.matmul(dmy_ps[:], dmy[:, 0:128], dmy[:],
                             start=True, stop=True)

        def filler(cols=256):
            nc.tensor.matmul(dmy_ps[:, 0:cols], dmy[:, 0:128], dmy[:, 0:cols],
                             start=True, stop=True)

        # Load the exp ACT table before steady state (Copy/Identity live in
        # every table, so this is the only table load).
        acttbl = cst.tile([128, 8], BF16, name="acttbl", tag="acttbl")
        nc.scalar.activation(out=acttbl[:], in_=dmy[:, 0:8], func=AF.Exp,
                             bias=0.0, scale=1.0)

        # ---------------- prologue ----------------
        # ktb[o,j] = sum_c wk[o,c] y[j,c] + bk[o]   (4K=128 j-replicas)
        kt_ps = ps.tile([C4, 4 * K], F32, name="kt_ps", tag="e", bufs=2)
        for k in range(KC):
            nc.tensor.matmul(kt_ps[:], wkt[:, k, :], yt[:, k, :],
                             start=(k == 0), stop=(k == KC - 1))
        ktb4 = cst.tile([C4, 4 * K], BF16, name="ktb4", tag="ktb4")
        nc.scalar.activation(out=ktb4[:], in_=kt_ps[:], func=AF.Identity,
                             bias=bk_sb[:], scale=1.0)

        # EM[c,j] = sum_o wq[o,c] ktb[o,j]  (c-tiled: [128, KC, 128])
        em_ps = ps.tile([128, KC, 128], F32, name="em_ps", tag="e", bufs=2)
        for k in range(KC):
            nc.tensor.matmul(em_ps[:, k, :], wqo[:, k * 128:(k + 1) * 128],
                             ktb4[:], start=True, stop=True)
        em_sb = cst.tile([128, KC, 128], BF16, name="em_sb", tag="em_sb")
        nc.scalar.activation(out=em_sb[:], in_=em_ps[:], func=AF.Copy,
                             bias=0.0, scale=1.0)

        # v[j,c] = sum_cl y[j,cl] wv[c,cl], scaled by s
        v_ps = ps.tile([K, C], F32, name="v_ps", tag="s", bufs=1)
        for k in range(KC):
            nc.tensor.matmul(v_ps[:], yt[:, k, 0:K], wvt[:, k, :],
                             start=(k == 0), stop=(k == KC - 1))
        v_sb = cst.tile([K, C], BF16, name="v_sb", tag="v_sb")
        nc.scalar.activation(out=v_sb[:], in_=v_ps[:], func=AF.Copy,
                             bias=0.0, scale=s_bc32[:])
        # partition-stacked copy for row-packed final matmuls:
        # vstack[32*ct + j, m] = v_sb[j, 128*ct + m]
        vstack = cst.tile([128, 128], BF16, name="vstack", tag="vstack")
        for ct in range(CT):
            nc.gpsimd.dma_start(
                vstack[32 * ct:32 * (ct + 1), :],
                v_sb[:, 128 * ct:128 * (ct + 1)])

        # e_b[j] = sum_o bq[o] ktb[o,j] -> exp bias, per partition
        eb_ps = ps.tile([4 * K, K], F32, name="eb_ps", tag="o", bufs=4)
        nc.tensor.matmul(eb_ps[:], ktb4[:], bqb[:], start=True, stop=True)
        e_b4 = cst.tile([4 * K, 1], F32, name="e_b4", tag="e_b4")
        nc.scalar.activation(out=e_b4[:], in_=eb_ps[:, 0:1], func=AF.Copy,
                             bias=0.0, scale=1.0)

        # s*bv per output row-tile: [C4, CT] f32
        bvs = cst.tile([C4, CT], F32, name="bvs", tag="bvs")
        nc.vector.tensor_scalar_mul(bvs[:], bvt_sb[:], s_bc128[:])

        # ------------- software-pipelined main loop over column chunks ----
        #   step i:  e/exp(i)   sum/rec/mul(i-1)   out-mm/copy/store(i-2)
        expts = [None] * NPC
        attns = [None] * NPC

        def stage_e(pc):
            e_ps = ps.tile([128, PC], F32, name=f"e_ps{pc}", tag="e", bufs=2)
            for k in range(KC):
                nc.tensor.matmul(e_ps[:], em_sb[:, k, :], xs[pc][:, k, :],
                                 start=(k == 0), stop=(k == KC - 1))
            expt = wk_pool.tile([128, PC], BF16, name="expt", tag="expt",
                                bufs=3)
            nc.scalar.activation(out=expt[:], in_=e_ps[:], func=AF.Exp,
                                 bias=e_b4[:], scale=1.0)
            expts[pc] = expt

        def stage_s(pc):
            s_ps = ps.tile([128, PC], F32, name=f"s_ps{pc}", tag="s", bufs=1)
            nc.tensor.matmul(s_ps[:], ones32[:], expts[pc][0:K, :],
                             start=True, stop=True)
            rec = wk_pool.tile([128, PC], F32, name="rec", tag="rec", bufs=2)
            nc.vector.reciprocal_approx_fast(out=rec[:], in_=s_ps[:])
            attn = wk_pool.tile([128, PC], BF16, name="attn", tag="attn",
                                bufs=3)
            nc.vector.tensor_tensor(attn[:], expts[pc][:], rec[:],
                                    op=AluOpType.mult)
            attns[pc] = attn

        def stage_out(pc):
            sl = slice(pc * PC, (pc + 1) * PC)
            xt = xs[pc]
            attn = attns[pc]
            osb = wk_pool.tile([128, CT, PC], BF16, name="osb", tag="osb",
                               bufs=3)
            for ct in range(CT):
                o_ps = ps.tile([128, PC], F32, name=f"o_ps{pc}_{ct}",
                               tag="o", bufs=4)
                # residual first: eye@x is ready before attn is
                nc.tensor.matmul(o_ps[:], eye[:], xt[:, ct, :],
                                 start=True, stop=False)
                nc.tensor.matmul(o_ps[:],
                                 vstack[32 * ct:32 * (ct + 1), :],
                                 attn[32 * ct:32 * (ct + 1), :],
                                 start=False, stop=True,
                                 tile_position=(32 * ct, 0))
                if ct < 2:
                    nc.scalar.activation(out=osb[:, ct, :], in_=o_ps[:],
                                         func=AF.Identity,
                                         bias=bvs[:, ct:ct + 1], scale=1.0)
                else:
                    nc.vector.tensor_scalar_add(osb[:, ct, :], o_ps[:],
                                                bvs[:, ct:ct + 1])
            dst = nc.t.out[:, sl].rearrange("(k p) f -> p k f", p=128)
            cdma.dma_start(dst, osb[:])

        for step in range(NPC + 3):
            if 1 <= step and step + 3 < NPC:
                load_chunk(step + 3)
            if step < NPC:
                filler()
                stage_e(step)
            if 0 <= step - 1 < NPC:
                stage_s(step - 1)
            if 0 <= step - 2 < NPC:
                filler()
                stage_out(step - 2)


class _T:
    """Attribute access to declared dram params."""
    def __init__(self):
        self.__dict__ = {}


_NC_CACHE = []


def _build():
    if _NC_CACHE:
        return _NC_CACHE[0]
    nc = bacc.Bacc(target_bir_lowering=False)
    nc.t = _T()
    t = nc.t
    t.x = nc.declare_dram_parameter("x", [C, HW], BF16, isOutput=False)
    t.yt = nc.declare_dram_parameter("yt", [C, 4 * K], BF16, isOutput=False)
    t.wqo = nc.declare_dram_parameter("wqo", [C4, C], BF16, isOutput=False)
    t.wkt = nc.declare_dram_parameter("wkt", [C, C4], BF16, isOutput=False)
    t.wvt = nc.declare_dram_parameter("wvt", [C, C], BF16, isOutput=False)
    t.eye = nc.declare_dram_parameter("eye", [128, 128], BF16, isOutput=False)
    t.bqb = nc.declare_dram_parameter("bqb", [C4, K], BF16, isOutput=False)
    t.bk = nc.declare_dram_parameter("bk", [C4, 1], F32, isOutput=False)
    t.bvt = nc.declare_dram_parameter("bvt", [C4, CT], F32, isOutput=False)
    t.s = nc.declare_dram_parameter("s", [1, 1], F32, isOutput=False)
    t.out = nc.declare_dram_parameter("out", [C, HW], BF16, isOutput=True)
    with tile.TileContext(nc) as tc:
        _emit(nc, tc)
    nc.finalize()
    _NC_CACHE.append(nc)
    return nc


def _in_maps(x, y, wq, bq, wk, bk, wv, bv, scale):
    x = np.ascontiguousarray(x, dtype=np.float32).reshape(N, C, HW).astype(BF)
    yt = np.ascontiguousarray(
        np.tile(np.transpose(y, (0, 2, 1)), (1, 1, 4))).astype(BF)
    wqo = np.ascontiguousarray(wq, dtype=np.float32).astype(BF)
    wkt = np.ascontiguousarray(wk.T, dtype=np.float32).astype(BF)
    wvt = np.ascontiguousarray(wv.T, dtype=np.float32).astype(BF)
    eye = np.eye(128, dtype=np.float32).astype(BF)
    bqb = np.ascontiguousarray(
        np.broadcast_to(np.float32(bq).reshape(C4, 1), (C4, K))).astype(BF)
    bk = np.ascontiguousarray(bk, dtype=np.float32).reshape(C4, 1)
    bvt = np.ascontiguousarray(
        np.float32(bv).reshape(CT, C4).T, dtype=np.float32)
    s = np.ascontiguousarray(scale, dtype=np.float32).reshape(1, 1)
    return [
        {
            "x": x[i], "yt": yt[i], "wqo": wqo, "wkt": wkt, "wvt": wvt,
            "eye": eye, "bqb": bqb, "bk": bk, "bvt": bvt, "s": s,
        }
        for i in range(N)
    ]


def _run(inputs, **kwargs):
    nc = _build()
    return run_bass_kernel_spmd(nc, _in_maps(**inputs),
                                core_ids=list(range(N)), **kwargs)


def kernel(**inputs) -> np.ndarray:
    res = _run(inputs)
    out = np.stack([np.asarray(res.results[i]["out"], dtype=np.float32)
                    for i in range(N)])
    return out.reshape(N, C, H, W)


# revision 6
# speedup vs baseline: 1.2992x; 1.1853x over previous
"""CPAMDec attention-decoder kernel for 8 Trainium2 NeuronCores.

Reference computation (per batch n of N=8):
    q  = x_n^T @ wq^T + bq          (HW=4096, C4=128)
    k  = y_n @ wk^T + bk            (K=32, C4=128)
    v  = y_n @ wv^T + bv            (K=32, C=512)
    attn = softmax(q @ k^T, axis=-1)        (HW, K)
    out = scale * (v^T @ attn^T) + x_n      (C, HW)

Sharding: pure data parallel - core i computes batch i; params replicated.

Key optimizations:
  - bf16 I/O. x and out move over HBM as bf16 (8MB -> 4MB each way per
    core); HBM-per-NC (~358 GB/s) is the binding roofline. rel-err of the
    full bf16 pipeline is ~3e-3, well under the 2e-2 gate.
  - wq folding: e[j,p] = sum_c EM[c,j] x[c,p] with EM = wq^T @ (k^T+bk)
    computed once in the prologue. The per-chunk q stage (4 matmuls + an
    ACT copy) disappears; energy comes straight from x.
  - bq contributes a per-key bias e_b[j] = sum_o bq[o]*ktb[o,j], applied
    inside the exp() activation (exact algebra).
  - bv enters as a per-partition scalar in the fused output STT
    osb = (o_ps + s*bv[c]) + x, using sum_j attn[p,j] = 1. The 4 STTs
    per chunk are split 2 on DVE / 2 on GpSimd(Pool).
  - Consts ride in 3 packed DRAM params (DMA issue costs ~0.7us of queue
    time each; 10 separate loads would serialize startup by ~7us).
  - PE warm-up dummies ramp the HAM clock gate while DMAs land.
"""

import sys

sys.path.insert(0, "/opt/trn_rl_repo")

import numpy as np
import ml_dtypes

import concourse.bacc as bacc
import concourse.mybir as mybir
import concourse.tile as tile
from concourse.alu_op_type import AluOpType
from concourse.bass_utils import run_bass_kernel_spmd

F32 = mybir.dt.float32
BF16 = mybir.dt.bfloat16
AF = mybir.ActivationFunctionType
BF = ml_dtypes.bfloat16

N, C, H, W, K = 8, 512, 64, 64, 32
HW = H * W            # 4096
C4 = C // 4           # 128
PC = 512              # free-dim chunk (1 PSUM bank of fp32)
NPC = HW // PC        # 8 chunks
KC = C // 128         # 4 contraction chunks
CT = C // 128         # 4 output row-tiles


def _emit(nc, tc):
    sync = nc.sync
    cdma = nc.scalar      # consts ride the ACT HWDGE ring

    with (
        tc.tile_pool(name="const", bufs=1) as cst,
        tc.tile_pool(name="xbuf", bufs=1) as xp,
        tc.tile_pool(name="work", bufs=3) as wk_pool,
        tc.tile_pool(name="ps", bufs=2, space="PSUM") as ps,
    ):
        # ---------------- constant loads (3 packed DMAs) ----------------
        # pa = wqo[128,512] | bqb[128,32]            bf16
        # pf = bk[128,1] | bvt[128,4]                f32
        # pb = wkt | yt | wvt  as [512, 768] -> [128, 4, 768]  bf16
        pa = cst.tile([128, C + K + C], BF16, name="pa", tag="pa")
        cdma.dma_start(pa[:], nc.t.pa[:])
        wqo = pa[:, 0:C]
        bqb = pa[:, C:C + K]
        bvb32 = pa[0:K, C + K:C + K + C]
        pf = cst.tile([128, 1], F32, name="pf", tag="pf")
        cdma.dma_start(pf[:], nc.t.pf[:])
        bk_sb = pf[:, 0:1]
        pb = cst.tile([128, KC, 128 + 128 + C], BF16, name="pb", tag="pb")
        cdma.dma_start(pb[:], nc.t.pb[:].rearrange("(k p) f -> p k f", p=128))

        def wkt(k):
            return pb[:, k, 0:C4]

        def yt(k):
            return pb[:, k, 128:128 + 4 * K]

        def wvt(k):
            return pb[:, k, 256:256 + C]

        ones32 = cst.tile([K, 128], BF16, name="ones32", tag="ones32")
        nc.gpsimd.memset(ones32[:], 1.0)

        # x column chunks: (128 part, 4 c-tiles, PC cols) strided loads on
        # the SP ring. SBUF is plentiful: keep all 8 resident.
        xs = [None] * NPC

        def load_chunk(pc):
            t = xp.tile([128, KC, PC], BF16, name=f"xs{pc}", tag=f"xs{pc}")
            src = nc.t.x[:, pc * PC:(pc + 1) * PC].rearrange(
                "(k p) f -> p k f", p=128)
            sync.dma_start(t[:], src)
            xs[pc] = t

        for pc in range(4):
            load_chunk(pc)

        # ---------------- PE warm-up ----------------
        # Ramp the HAM clock gate (1.2 -> 2.4 GHz after ~3.4us sustained)
        # while DMAs land. Reads pa (first const to arrive).
        dmy_ps = ps.tile([128, PC], F32, name="dmy_ps", tag="s", bufs=1)
        for _ in range(7):
            nc.tensor.matmul(dmy_ps[:], pa[:, 0:128], wqo[:],
                             start=True, stop=True)

        # Load the exp ACT table before steady state (Copy/Identity live in
        # every table, so this is the only table load).
        acttbl = cst.tile([128, 8], BF16, name="acttbl", tag="acttbl")
        nc.scalar.activation(out=acttbl[:], in_=pa[:, 0:8], func=AF.Exp,
                             bias=0.0, scale=1.0)

        # ---------------- prologue ----------------
        # ktb[o,j] = sum_c wk[o,c] y[j,c] + bk[o]   (4K=128 j-replicas)
        kt_ps = ps.tile([C4, 4 * K], F32, name="kt_ps", tag="e", bufs=2)
        for k in range(KC):
            nc.tensor.matmul(kt_ps[:], wkt(k), yt(k),
                             start=(k == 0), stop=(k == KC - 1))
        ktb4 = cst.tile([C4, 4 * K], BF16, name="ktb4", tag="ktb4")
        nc.scalar.activation(out=ktb4[:], in_=kt_ps[:], func=AF.Identity,
                             bias=bk_sb, scale=1.0)

        # EM[c,j] = sum_o wq[o,c] ktb[o,j]  (c-tiled: [128, KC, 128])
        em_ps = ps.tile([128, KC, 128], F32, name="em_ps", tag="e", bufs=2)
        for k in range(KC):
            nc.tensor.matmul(em_ps[:, k, :], wqo[:, k * 128:(k + 1) * 128],
                             ktb4[:], start=True, stop=True)
        em_sb = cst.tile([128, KC, 128], BF16, name="em_sb", tag="em_sb")
        nc.scalar.activation(out=em_sb[:], in_=em_ps[:], func=AF.Copy,
                             bias=0.0, scale=1.0)

        # v[j,c] = sum_cl y[j,cl] wv[c,cl], scaled by s
        v_ps = ps.tile([K, C], F32, name="v_ps", tag="s", bufs=1)
        for k in range(KC):
            nc.tensor.matmul(v_ps[:], yt(k)[:, 0:K], wvt(k),
                             start=(k == 0), stop=(k == KC - 1))
        v_sb = cst.tile([K, C], BF16, name="v_sb", tag="v_sb")
        nc.vector.tensor_tensor(v_sb[:], v_ps[:], bvb32,
                                op=AluOpType.add)
        # partition-stacked copy for row-packed final matmuls:
        # vstack[32*ct + j, m] = v_sb[j, 128*ct + m]
        vstack = cst.tile([128, 128], BF16, name="vstack", tag="vstack")
        for ct in range(CT):
            nc.gpsimd.dma_start(
                vstack[32 * ct:32 * (ct + 1), :],
                v_sb[:, 128 * ct:128 * (ct + 1)])

        # e_b[j] = sum_o bq[o] ktb[o,j] -> exp bias, per partition
        eb_ps = ps.tile([4 * K, K], F32, name="eb_ps", tag="o", bufs=4)
        nc.tensor.matmul(eb_ps[:], ktb4[:], bqb[:], start=True, stop=True)
        e_b4 = cst.tile([4 * K, 1], F32, name="e_b4", tag="e_b4")
        nc.scalar.activation(out=e_b4[:], in_=eb_ps[:, 0:1], func=AF.Copy,
                             bias=0.0, scale=1.0)

        # ------------- software-pipelined main loop over column chunks ----
        #   step i:  e/exp(i)   sum/rec/mul(i-1)   out-mm/stt/store(i-2)
        expts = [None] * NPC
        attns = [None] * NPC

        def stage_e(pc):
            e_ps = ps.tile([128, PC], F32, name=f"e_ps{pc}", tag="e", bufs=2)
            for k in range(KC):
                nc.tensor.matmul(e_ps[:], em_sb[:, k, :], xs[pc][:, k, :],
                                 start=(k == 0), stop=(k == KC - 1))
            expt = wk_pool.tile([128, PC], BF16, name="expt", tag="expt",
                                bufs=3)
            nc.scalar.activation(out=expt[:], in_=e_ps[:], func=AF.Exp,
                                 bias=e_b4[:], scale=1.0)
            expts[pc] = expt

        def stage_s(pc):
            s_ps = ps.tile([128, PC], F32, name=f"s_ps{pc}", tag="s", bufs=1)
            nc.tensor.matmul(s_ps[:], ones32[:], expts[pc][0:K, :],
                             start=True, stop=True)
            rec = wk_pool.tile([128, PC], F32, name="rec", tag="rec", bufs=2)
            nc.vector.reciprocal_approx_fast(out=rec[:], in_=s_ps[:])
            attn = wk_pool.tile([128, PC], BF16, name="attn", tag="attn",
                                bufs=3)
            nc.gpsimd.tensor_tensor(attn[:], expts[pc][:], rec[:],
                                    op=AluOpType.mult)
            attns[pc] = attn

        def stage_out(pc):
            sl = slice(pc * PC, (pc + 1) * PC)
            xt = xs[pc]
            attn = attns[pc]
            osb = wk_pool.tile([128, CT, PC], BF16, name="osb", tag="osb",
                               bufs=3)
            for ct in range(CT):
                o_ps = ps.tile([128, PC], F32, name=f"o_ps{pc}_{ct}",
                               tag="o", bufs=4)
                nc.tensor.matmul(o_ps[:],
                                 vstack[32 * ct:32 * (ct + 1), :],
                                 attn[32 * ct:32 * (ct + 1), :],
                                 start=True, stop=True,
                                 tile_position=(32 * ct, 0))
                nc.vector.tensor_tensor(osb[:, ct, :], o_ps[:],
                                        xt[:, ct, :], op=AluOpType.add)
            dst = nc.t.out[:, sl].rearrange("(k p) f -> p k f", p=128)
            sync.dma_start(dst, osb[:])

        for step in range(NPC + 3):
            if 1 <= step and step + 3 < NPC:
                load_chunk(step + 3)
            if step < NPC:
                stage_e(step)
            if 0 <= step - 1 < NPC:
                stage_s(step - 1)
            if 0 <= step - 2 < NPC:
                stage_out(step - 2)


class _T:
    """Attribute access to declared dram params."""
    def __init__(self):
        self.__dict__ = {}


_NC_CACHE = []


def _build():
    if _NC_CACHE:
        return _NC_CACHE[0]
    nc = bacc.Bacc(target_bir_lowering=False)
    nc.t = _T()
    t = nc.t
    t.x = nc.declare_dram_parameter("x", [C, HW], BF16, isOutput=False)
    t.pa = nc.declare_dram_parameter("pa", [128, C + K + C], BF16,
                                     isOutput=False)
    t.pf = nc.declare_dram_parameter("pf", [128, 1], F32, isOutput=False)
    t.pb = nc.declare_dram_parameter("pb", [C, 256 + C], BF16, isOutput=False)
    t.out = nc.declare_dram_parameter("out", [C, HW], BF16, isOutput=True)
    with tile.TileContext(nc) as tc:
        _emit(nc, tc)
    nc.finalize()
    _NC_CACHE.append(nc)
    return nc


def _in_maps(x, y, wq, bq, wk, bk, wv, bv, scale):
    x = np.ascontiguousarray(x, dtype=np.float32).reshape(N, C, HW).astype(BF)
    yt = np.ascontiguousarray(
        np.tile(np.transpose(y, (0, 2, 1)), (1, 1, 4))).astype(BF)
    s = float(np.float32(scale).reshape(-1)[0])
    wqo = np.ascontiguousarray(wq, dtype=np.float32).astype(BF)
    wkt = np.ascontiguousarray(wk.T, dtype=np.float32).astype(BF)
    wvt = np.ascontiguousarray(wv.T * s, dtype=np.float32).astype(BF)
    bqb = np.ascontiguousarray(
        np.broadcast_to(np.float32(bq).reshape(C4, 1), (C4, K))).astype(BF)
    bvb = np.zeros((128, C), dtype=BF)
    bvb[0:K, :] = np.float32(bv).reshape(1, C) * s
    bkc = np.ascontiguousarray(bk, dtype=np.float32).reshape(C4, 1)
    pa = np.concatenate([wqo, bqb, bvb], axis=1)            # [128, 1056]
    return [
        {
            "x": x[i],
            "pa": pa,
            "pf": bkc,
            "pb": np.concatenate([wkt, yt[i], wvt], axis=1),
        }
        for i in range(N)
    ]


def _run(inputs, **kwargs):
    nc = _build()
    return run_bass_kernel_spmd(nc, _in_maps(**inputs),
                                core_ids=list(range(N)), **kwargs)


def kernel(**inputs) -> np.ndarray:
    res = _run(inputs)
    out = np.stack([np.asarray(res.results[i]["out"], dtype=np.float32)
                    for i in range(N)])
    return out.reshape(N, C, H, W)


# revision 8
# speedup vs baseline: 1.5715x; 1.2097x over previous
"""CPAMDec attention-decoder kernel for 8 Trainium2 NeuronCores.

Reference computation (per batch n of N=8):
    q  = x_n^T @ wq^T + bq          (HW=4096, C4=128)
    k  = y_n @ wk^T + bk            (K=32, C4=128)
    v  = y_n @ wv^T + bv            (K=32, C=512)
    attn = softmax(q @ k^T, axis=-1)        (HW, K)
    out = scale * (v^T @ attn^T) + x_n      (C, HW)

Sharding: pure data parallel - core i computes batch i; params replicated.

Key optimizations:
  - bf16 I/O. x and out move over HBM as bf16 (8MB -> 4MB each way per
    core); HBM-per-NC (~358 GB/s) is the binding roofline. rel-err of the
    full bf16 pipeline is ~3e-3, well under the 2e-2 gate.
  - wq folding: e[j,p] = sum_c EM[c,j] x[c,p] with EM = wq^T @ (k^T+bk)
    computed once in the prologue. The per-chunk q stage (4 matmuls + an
    ACT copy) disappears; energy comes straight from x.
  - bq contributes a per-key bias e_b[j] = sum_o bq[o]*ktb[o,j], applied
    inside the exp() activation (exact algebra).
  - bv enters as a per-partition scalar in the fused output STT
    osb = (o_ps + s*bv[c]) + x, using sum_j attn[p,j] = 1. The 4 STTs
    per chunk are split 2 on DVE / 2 on GpSimd(Pool).
  - Consts ride in 3 packed DRAM params (DMA issue costs ~0.7us of queue
    time each; 10 separate loads would serialize startup by ~7us).
  - PE warm-up dummies ramp the HAM clock gate while DMAs land.
"""

import sys

sys.path.insert(0, "/opt/trn_rl_repo")

import numpy as np
import ml_dtypes

import concourse.bacc as bacc
import concourse.mybir as mybir
import concourse.tile as tile
from concourse.alu_op_type import AluOpType
from concourse.bass_utils import run_bass_kernel_spmd

F32 = mybir.dt.float32
BF16 = mybir.dt.bfloat16
AF = mybir.ActivationFunctionType
BF = ml_dtypes.bfloat16

N, C, H, W, K = 8, 512, 64, 64, 32
HW = H * W            # 4096
C4 = C // 4           # 128
PC = 512              # free-dim chunk (1 PSUM bank of fp32)
NPC = HW // PC        # 8 chunks
KC = C // 128         # 4 contraction chunks
CT = C // 128         # 4 output row-tiles


def _emit(nc, tc):
    sync = nc.sync
    cdma = nc.scalar      # consts ride the ACT HWDGE ring

    with (
        tc.tile_pool(name="const", bufs=1) as cst,
        tc.tile_pool(name="xbuf", bufs=1) as xp,
        tc.tile_pool(name="work", bufs=3) as wk_pool,
        tc.tile_pool(name="ps", bufs=2, space="PSUM") as ps,
    ):
        # ---------------- constant loads (3 packed DMAs) ----------------
        # pa = wqo[128,512] | bqb[128,32]            bf16
        # pf = bk[128,1] | bvt[128,4]                f32
        # pb = wkt | yt | wvt  as [512, 768] -> [128, 4, 768]  bf16
        pa = cst.tile([128, C + K + C], BF16, name="pa", tag="pa")
        cdma.dma_start(pa[:], nc.t.pa[:])
        wqo = pa[:, 0:C]
        bqb = pa[:, C:C + K]
        bvb32 = pa[0:K, C + K:C + K + C]
        pf = cst.tile([128, 1], F32, name="pf", tag="pf")
        cdma.dma_start(pf[:], nc.t.pf[:])
        bk_sb = pf[:, 0:1]
        pb = cst.tile([128, KC, 128 + 128 + C], BF16, name="pb", tag="pb")
        cdma.dma_start(pb[:], nc.t.pb[:].rearrange("(k p) f -> p k f", p=128))

        def wkt(k):
            return pb[:, k, 0:C4]

        def yt(k):
            return pb[:, k, 128:128 + 4 * K]

        def wvt(k):
            return pb[:, k, 256:256 + C]

        ones32 = cst.tile([K, 128], BF16, name="ones32", tag="ones32")
        nc.gpsimd.memset(ones32[:], 1.0)

        # x column chunks: (128 part, 4 c-tiles, PC cols) strided loads on
        # the SP ring. SBUF is plentiful: keep all 8 resident.
        xs = [None] * NPC

        def load_chunk(pc):
            t = xp.tile([128, KC, PC], BF16, name=f"xs{pc}", tag=f"xs{pc}")
            src = nc.t.x[:, pc * PC:(pc + 1) * PC].rearrange(
                "(k p) f -> p k f", p=128)
            sync.dma_start(t[:], src)
            xs[pc] = t

        for pc in range(4):
            load_chunk(pc)

        # ---------------- PE warm-up ----------------
        # Ramp the HAM clock gate (1.2 -> 2.4 GHz after ~3.4us sustained)
        # while DMAs land. Reads pa (first const to arrive).
        dmy_ps = ps.tile([128, PC], F32, name="dmy_ps", tag="s", bufs=1)
        for _ in range(7):
            nc.tensor.matmul(dmy_ps[:], pa[:, 0:128], wqo[:],
                             start=True, stop=True)

        # Load the exp ACT table before steady state (Copy/Identity live in
        # every table, so this is the only table load).
        acttbl = cst.tile([128, 8], BF16, name="acttbl", tag="acttbl")
        nc.scalar.activation(out=acttbl[:], in_=pa[:, 0:8], func=AF.Exp,
                             bias=0.0, scale=1.0)

        # ---------------- prologue ----------------
        # ktb[o,j] = sum_c wk[o,c] y[j,c] + bk[o]   (4K=128 j-replicas)
        kt_ps = ps.tile([C4, 4 * K], F32, name="kt_ps", tag="e", bufs=2)
        for k in range(KC):
            nc.tensor.matmul(kt_ps[:], wkt(k), yt(k),
                             start=(k == 0), stop=(k == KC - 1))
        ktb4 = cst.tile([C4, 4 * K], BF16, name="ktb4", tag="ktb4")
        nc.scalar.activation(out=ktb4[:], in_=kt_ps[:], func=AF.Identity,
                             bias=bk_sb, scale=1.0)

        # EM[c,j] = sum_o wq[o,c] ktb[o,j]  (c-tiled: [128, KC, 128])
        em_ps = ps.tile([128, KC, 128], F32, name="em_ps", tag="e", bufs=2)
        for k in range(KC):
            nc.tensor.matmul(em_ps[:, k, :], wqo[:, k * 128:(k + 1) * 128],
                             ktb4[:], start=True, stop=True)
        em_sb = cst.tile([128, KC, 128], BF16, name="em_sb", tag="em_sb")
        nc.scalar.activation(out=em_sb[:], in_=em_ps[:], func=AF.Copy,
                             bias=0.0, scale=1.0)

        # v[j,c] = sum_cl y[j,cl] wv[c,cl], scaled by s
        v_ps = ps.tile([K, C], F32, name="v_ps", tag="s", bufs=1)
        for k in range(KC):
            nc.tensor.matmul(v_ps[:], yt(k)[:, 0:K], wvt(k),
                             start=(k == 0), stop=(k == KC - 1))
        v_sb = cst.tile([K, C], BF16, name="v_sb", tag="v_sb")
        nc.vector.tensor_tensor(v_sb[:], v_ps[:], bvb32,
                                op=AluOpType.add)
        # partition-stacked copy for row-packed final matmuls:
        # vstack[32*ct + j, m] = v_sb[j, 128*ct + m]
        vstack = cst.tile([128, 128], BF16, name="vstack", tag="vstack")
        for ct in range(CT):
            nc.gpsimd.dma_start(
                vstack[32 * ct:32 * (ct + 1), :],
                v_sb[:, 128 * ct:128 * (ct + 1)])

        # e_b[j] = sum_o bq[o] ktb[o,j] -> exp bias, per partition
        eb_ps = ps.tile([4 * K, K], F32, name="eb_ps", tag="o", bufs=2)
        nc.tensor.matmul(eb_ps[:], ktb4[:], bqb[:], start=True, stop=True)
        e_b4 = cst.tile([4 * K, 1], F32, name="e_b4", tag="e_b4")
        nc.scalar.activation(out=e_b4[:], in_=eb_ps[:, 0:1], func=AF.Copy,
                             bias=0.0, scale=1.0)

        # ------------- software-pipelined main loop over column chunks ----
        #   step i:  e/exp(i)   sum/rec/mul(i-1)   out-mm/stt/store(i-2)
        expts = [None] * NPC
        attns = [None] * NPC

        def stage_e(pc):
            e_ps = ps.tile([128, PC], F32, name=f"e_ps{pc}", tag="e", bufs=2)
            for k in range(KC):
                nc.tensor.matmul(e_ps[:], em_sb[:, k, :], xs[pc][:, k, :],
                                 start=(k == 0), stop=(k == KC - 1))
            expt = wk_pool.tile([128, PC], BF16, name="expt", tag="expt",
                                bufs=3)
            nc.scalar.activation(out=expt[:], in_=e_ps[:], func=AF.Exp,
                                 bias=e_b4[:], scale=1.0)
            expts[pc] = expt

        def stage_s(pc):
            s_ps = ps.tile([128, PC], F32, name=f"s_ps{pc}", tag="s", bufs=1)
            nc.tensor.matmul(s_ps[:], ones32[:], expts[pc][0:K, :],
                             start=True, stop=True)
            rec = wk_pool.tile([128, PC], F32, name="rec", tag="rec", bufs=2)
            nc.vector.reciprocal_approx_fast(out=rec[:], in_=s_ps[:])
            attn = wk_pool.tile([128, PC], BF16, name="attn", tag="attn",
                                bufs=3)
            nc.gpsimd.tensor_tensor(attn[:], expts[pc][:], rec[:],
                                    op=AluOpType.mult)
            attns[pc] = attn

        def stage_out(pc):
            xt = xs[pc]
            attn = attns[pc]
            osb = wk_pool.tile([128, CT, PC], BF16, name="osb", tag="osb",
                               bufs=3)
            # two double-bank PSUM tiles; one 1024-col DVE add per pair
            for h in range(2):
                o_ps = ps.tile([128, 2, PC], F32, name=f"o_ps{pc}_{h}",
                               tag="o", bufs=2)
                for i in range(2):
                    ct = 2 * h + i
                    nc.tensor.matmul(o_ps[:, i, :],
                                     vstack[32 * ct:32 * (ct + 1), :],
                                     attn[32 * ct:32 * (ct + 1), :],
                                     start=True, stop=True,
                                     tile_position=(32 * ct, 0))
                nc.vector.tensor_tensor(osb[:, 2 * h:2 * h + 2, :], o_ps[:],
                                        xt[:, 2 * h:2 * h + 2, :],
                                        op=AluOpType.add)
                dst = nc.t.out[2 * h * 128:(2 * h + 2) * 128,
                               pc * PC:(pc + 1) * PC].rearrange(
                    "(k p) f -> p k f", p=128)
                sync.dma_start(dst, osb[:, 2 * h:2 * h + 2, :])

        for step in range(NPC + 3):
            if 1 <= step and step + 3 < NPC:
                load_chunk(step + 3)
            if step < NPC:
                stage_e(step)
            if 0 <= step - 1 < NPC:
                stage_s(step - 1)
            if 0 <= step - 2 < NPC:
                stage_out(step - 2)


class _T:
    """Attribute access to declared dram params."""
    def __init__(self):
        self.__dict__ = {}


_NC_CACHE = []


def _build():
    if _NC_CACHE:
        return _NC_CACHE[0]
    nc = bacc.Bacc(target_bir_lowering=False)
    nc.t = _T()
    t = nc.t
    t.x = nc.declare_dram_parameter("x", [C, HW], BF16, isOutput=False)
    t.pa = nc.declare_dram_parameter("pa", [128, C + K + C], BF16,
                                     isOutput=False)
    t.pf = nc.declare_dram_parameter("pf", [128, 1], F32, isOutput=False)
    t.pb = nc.declare_dram_parameter("pb", [C, 256 + C], BF16, isOutput=False)
    t.out = nc.declare_dram_parameter("out", [C, HW], BF16, isOutput=True)
    with tile.TileContext(nc) as tc:
        _emit(nc, tc)
    nc.finalize()
    _NC_CACHE.append(nc)
    return nc


def _in_maps(x, y, wq, bq, wk, bk, wv, bv, scale):
    x = np.ascontiguousarray(x, dtype=np.float32).reshape(N, C, HW).astype(BF)
    yt = np.ascontiguousarray(
        np.tile(np.transpose(y, (0, 2, 1)), (1, 1, 4))).astype(BF)
    s = float(np.float32(scale).reshape(-1)[0])
    wqo = np.ascontiguousarray(wq, dtype=np.float32).astype(BF)
    wkt = np.ascontiguousarray(wk.T, dtype=np.float32).astype(BF)
    wvt = np.ascontiguousarray(wv.T * s, dtype=np.float32).astype(BF)
    bqb = np.ascontiguousarray(
        np.broadcast_to(np.float32(bq).reshape(C4, 1), (C4, K))).astype(BF)
    bvb = np.zeros((128, C), dtype=BF)
    bvb[0:K, :] = np.float32(bv).reshape(1, C) * s
    bkc = np.ascontiguousarray(bk, dtype=np.float32).reshape(C4, 1)
    pa = np.concatenate([wqo, bqb, bvb], axis=1)            # [128, 1056]
    return [
        {
            "x": x[i],
            "pa": pa,
            "pf": bkc,
            "pb": np.concatenate([wkt, yt[i], wvt], axis=1),
        }
        for i in range(N)
    ]


def _run(inputs, **kwargs):
    nc = _build()
    return run_bass_kernel_spmd(nc, _in_maps(**inputs),
                                core_ids=list(range(N)), **kwargs)


def kernel(**inputs) -> np.ndarray:
    res = _run(inputs)
    out = np.stack([np.asarray(res.results[i]["out"], dtype=np.float32)
                    for i in range(N)])
    return out.reshape(N, C, H, W)
